# revision 3
# baseline (speedup 1.0000x reference)
"""MoE FFN (grouped top-1 routing, SwiGLU experts) on 8 Trainium2 NeuronCores.

Strategy (expert-parallel with static 2-segment load balancing):
  - Host computes the (tiny) routers in float64: sigmoid(x @ macro_w) -> top-1
    group of 4; within the selected group both 2 experts are active
    (TOP_K == EXPERTS_PER_GROUP) with sigmoid-normalized weights.
  - Each core processes C = s0 + s1 token columns in two statically-sized
    segments.  Each segment has its own full SwiGLU weight-set input, so the
    host can assign ANY expert to any (core, segment) bucket.  A small search
    picks (s0, s1) so the 8 expert token-loads pack into the 16 buckets with
    minimal C (544 vs 608 for naive per-expert capacity on the benchmark
    routing distribution).
  - The per-token routing weight is applied on the HOST to the fp32 partial
    outputs (y[t] = w0*y_e0[t] + w1*y_e1[t]), so the device never sees it and
    x is shipped only once.
  - Device kernel per segment: Y^T = down^T @ (silu(gate^T X^T) * (up^T X^T)),
    features on SBUF partitions, tokens on the free dim, bf16 storage/matmuls,
    fp32 PSUM accumulation.  All weight DMAs are issued in exact consumption
    order on the sync HWDGE queue (x first) so the PE never starves and the
    HAM clock-gate warms once.
"""

import numpy as np
import ml_dtypes

import concourse.bass as bass  # noqa: F401  (bass types via bacc)
import concourse.mybir as mybir
import concourse.tile as tile
from concourse import bacc
from concourse.bass_utils import run_bass_kernel_spmd

P = 128
D_MODEL = 1024
FFN_DIM = 2048
NUM_EXPERTS = 8
NUM_GROUPS = 4
EPS = 1e-9
DO = D_MODEL // P   # 8 k-tiles over D
FO = FFN_DIM // P   # 16 f-tiles over F

F32 = mybir.dt.float32
BF16 = mybir.dt.bfloat16

N_CORES = 8

_BUILD_CACHE: dict[tuple, object] = {}
_PLAN_CACHE: dict[tuple, tuple] = {}
LAST_RESULTS = None  # stashed BassKernelResults for test harnesses


# ──────────────────────────────────────────────────────────────────────
# Device program
# ──────────────────────────────────────────────────────────────────────

def _build(s0: int, s1: int, nsets: int = 2):
    """Bass/Tile program: C=s0+s1 token columns, two segments, each with its
    own full SwiGLU weight set.  Segment A runs fully (gate/up then down),
    then segment B — so the weight stream only has to feed one segment's
    demand at a time (~250 GB/s vs the ~358 GB/s per-core HBM cap) and the
    PE never starves after the start transient."""
    C = s0 + s1
    segs = [(0, s0), (s0, s1)]

    nc = bacc.Bacc(
        "TRN2",
        target_bir_lowering=False,
        debug=False,
        enable_asserts=False,
        num_devices=N_CORES,
    )
    xt = nc.dram_tensor("xt", [D_MODEL, C], BF16, kind="ExternalInput").ap()
    gws, uws, dws = [], [], []
    for k in range(2):
        gws.append(nc.dram_tensor(f"gw{k}", [D_MODEL, FFN_DIM], BF16,
                                  kind="ExternalInput").ap())
        uws.append(nc.dram_tensor(f"uw{k}", [D_MODEL, FFN_DIM], BF16,
                                  kind="ExternalInput").ap())
        dws.append(nc.dram_tensor(f"dw{k}", [FFN_DIM, D_MODEL], BF16,
                                  kind="ExternalInput").ap())
    yt = nc.dram_tensor("yt", [D_MODEL, C], F32, kind="ExternalOutput").ap()

    gwr = [g.rearrange("(do p) f -> p do f", p=P) for g in gws]
    uwr = [u.rearrange("(do p) f -> p do f", p=P) for u in uws]
    dwr = [w.rearrange("(fo p) d -> p fo d", p=P) for w in dws]
    xtr = xt.rearrange("(do p) c -> p do c", p=P)

    # gate/up f-blocks (start_fo, n_fo): segment A leads with tiny blocks so
    # the PE can start as soon as possible; B uses uniform wide blocks.
    blocksA = [(0, 1), (1, 1), (2, 2), (4, 4), (8, 4), (12, 4)]
    blocksB = [(0, 4), (4, 4), (8, 4), (12, 4)]
    ND = 2  # d-tiles per down-weight block

    with tile.TileContext(nc) as tc:
        with (
            tc.tile_pool(name="xp", bufs=1) as xp,
            tc.tile_pool(name="wp", bufs=4) as wp,
            tc.tile_pool(name="dp", bufs=4) as dp,
            tc.tile_pool(name="hp", bufs=1) as hp,
            tc.tile_pool(name="sp", bufs=4) as sp,
            tc.tile_pool(name="yp", bufs=4) as yp,
            tc.tile_pool(name="pgu", bufs=6, space="PSUM") as pgu,
            tc.tile_pool(name="pd", bufs=2, space="PSUM") as pd,
        ):
            xts = xp.tile([P, DO, C], BF16, tag="xt")
            gts, uts, dts = {}, {}, {}

            def load_gu(k, blocks, engine):
                for b, (sfo, nfo) in enumerate(blocks):
                    fsl = slice(sfo * P, (sfo + nfo) * P)
                    g = wp.tile([P, DO, nfo * P], BF16, tag="gt", name=f"g{k}_{b}")
                    engine(b).dma_start(g[:], gwr[k][:, :, fsl])
                    u = wp.tile([P, DO, nfo * P], BF16, tag="ut", name=f"u{k}_{b}")
                    engine(b).dma_start(u[:], uwr[k][:, :, fsl])
                    gts[(k, b)] = g
                    uts[(k, b)] = u

            def load_dw(k):
                for db in range(DO // ND):
                    dsl = slice(db * ND * P, (db + 1) * ND * P)
                    dt_ = dp.tile([P, FO, ND * P], BF16, tag="dt", name=f"d{k}_{db}")
                    nc.sync.dma_start(dt_[:], dwr[k][:, :, dsl])
                    dts[(k, db)] = dt_

            # Early small transfers on the scalar HWDGE ring (free ~3.5us,
            # before the sync ring finishes the framework preamble):
            # x for segment A + the first tiny gate/up blocks of A.
            nc.scalar.dma_start(xts[:, :, 0:s0], xtr[:, :, 0:s0])
            load_gu(0, blocksA, lambda b: nc.scalar if b < 2 else nc.sync)
            # Rest of the stream in exact consumption order on the sync ring.
            load_dw(0)
            nc.sync.dma_start(xts[:, :, s0:C], xtr[:, :, s0:C])
            load_gu(1, blocksB, lambda b: nc.sync)
            load_dw(1)

            hs = hp.tile([P, FO, C], BF16, tag="h")

            for si, (off, slen) in enumerate(segs):
                k = si
                csl = slice(off, off + slen)
                blocks = blocksA if si == 0 else blocksB
                fo2blk = {}
                for b, (sfo, nfo) in enumerate(blocks):
                    for fo in range(sfo, sfo + nfo):
                        fo2blk[fo] = b

                # gate/up for this segment
                for fo in range(FO):
                    b = fo2blk[fo]
                    fl = fo - blocks[b][0]
                    fsl = slice(fl * P, (fl + 1) * P)
                    gt4 = gts[(k, b)]
                    ut4 = uts[(k, b)]
                    psg = pgu.tile([P, slen], F32, tag="ps", name=f"psg_{fo}_{si}")
                    psu = pgu.tile([P, slen], F32, tag="ps", name=f"psu_{fo}_{si}")
                    for do in range(DO):
                        nc.tensor.matmul(
                            psg[:], gt4[:, do, fsl], xts[:, do, csl],
                            start=(do == 0), stop=(do == DO - 1),
                        )
                    for do in range(DO):
                        nc.tensor.matmul(
                            psu[:], ut4[:, do, fsl], xts[:, do, csl],
                            start=(do == 0), stop=(do == DO - 1),
                        )
                    sg = sp.tile([P, slen], F32, tag="sg")
                    nc.scalar.activation(
                        sg[:], psg[:], mybir.ActivationFunctionType.Silu
                    )
                    nc.vector.tensor_mul(out=hs[:, fo, csl], in0=sg[:], in1=psu[:])

                # down for this segment
                for db in range(DO // ND):
                    for half in range(ND):
                        do = db * ND + half
                        dsl = slice(half * P, (half + 1) * P)
                        psy = pd.tile([P, slen], F32, tag="psy",
                                      name=f"psy_{do}_{si}")
                        for fo in range(FO):
                            nc.tensor.matmul(
                                psy[:], dts[(k, db)][:, fo, dsl], hs[:, fo, csl],
                                start=(fo == 0), stop=(fo == FO - 1),
                            )
                        yo = yp.tile([P, slen], F32, tag="yo")
                        nc.any.tensor_copy(out=yo[:], in_=psy[:])
                        nc.gpsimd.dma_start(yt[do * P:(do + 1) * P, csl], yo[:])
    nc.finalize()
    return nc


def _get_program(s0: int, s1: int, nsets: int = 2):
    key = (s0, s1)
    if key not in _BUILD_CACHE:
        _BUILD_CACHE[key] = _build(s0, s1)
    return _BUILD_CACHE[key]


# ──────────────────────────────────────────────────────────────────────
# Host routing
# ──────────────────────────────────────────────────────────────────────

def _sigmoid(z):
    return 1.0 / (1.0 + np.exp(-z))


def _route(xf32, macro_w, micro_w):
    """Host routers in float64. Returns group index per token and per-token
    weights for the 2 experts of the selected group (float32)."""
    xf = xf32.astype(np.float64)
    ms = _sigmoid(xf @ macro_w.astype(np.float64))  # [T, G]
    g_sel = np.argmax(ms, axis=1)
    T = xf.shape[0]
    mval = ms[np.arange(T), g_sel]
    mv = mval / (mval + EPS)

    w2 = np.zeros((T, 2), np.float64)
    for g in range(NUM_GROUPS):
        idx = np.nonzero(g_sel == g)[0]
        if idx.size == 0:
            continue
        s = _sigmoid(xf[idx] @ micro_w[g].astype(np.float64))  # [n, 2]
        denom = s[:, 0] + s[:, 1] + EPS
        w2[idx, 0] = mv[idx] * s[:, 0] / denom
        w2[idx, 1] = mv[idx] * s[:, 1] / denom
    return g_sel, w2.astype(np.float32)


# ──────────────────────────────────────────────────────────────────────
# Segment-size search + bucket assignment
# ──────────────────────────────────────────────────────────────────────

def _feasible(n_e, s0, s1):
    """Can loads n_e pack into 8 buckets of s0 and 8 of s1 (each bucket a
    single expert)?  Returns per-expert (a, b) bucket counts or None."""
    cands = []
    for n in n_e:
        cc = []
        if n == 0:
            cc.append((0, 0))
        else:
            for a in range(9):
                rem = n - a * s0
                b = 0 if rem <= 0 else -(-rem // s1)
                if b <= 8:
                    cc.append((a, b))
            # prune dominated
            cc.sort()
            pruned, best_b = [], 99
            for a, b in cc:
                if b < best_b:
                    pruned.append((a, b))
                    best_b = b
            cc = pruned
        cands.append(cc)
    # DP over (sum_a, sum_b)
    states = {(0, 0): []}
    for cc in cands:
        nxt = {}
        for (ua, ub), hist in states.items():
            for a, b in cc:
                na, nb = ua + a, ub + b
                if na <= 8 and nb <= 8 and (na, nb) not in nxt:
                    nxt[(na, nb)] = hist + [(a, b)]
        states = nxt
        if not states:
            return None
    return next(iter(states.values()))


def _plan(n_e):
    """Pick (s0, s1, nsets, ab) minimizing C = s0 + s1."""
    key = tuple(n_e)
    if key in _PLAN_CACHE:
        return _PLAN_CACHE[key]
    best = None
    for s0 in range(64, 513, 16):
        for s1 in range(48, s0 + 1, 16):
            ab = _feasible(n_e, s0, s1)
            if ab is None:
                continue
            c = s0 + s1
            if best is None or c < best[0] or (c == best[0] and s0 < best[1]):
                best = (c, s0, s1, ab)
    if best is None:
        raise RuntimeError(f"no feasible segment plan for loads {n_e}")
    _, s0, s1, ab = best
    # single-expert-per-core case needs only one weight set
    nsets = 2
    if all(a + b <= (1 if n else 99) for (a, b), n in zip(ab, n_e)) and False:
        nsets = 1  # (kept for experiments; default always 2)
    plan = (s0, s1, nsets, ab)
    _PLAN_CACHE[key] = plan
    return plan


# ──────────────────────────────────────────────────────────────────────
# Entry point
# ──────────────────────────────────────────────────────────────────────

def kernel(x, macro_w, micro_w, gate_w, up_w, down_w):
    global LAST_RESULTS
    x = np.asarray(x)
    B, S, D = x.shape
    T = B * S
    xf = np.ascontiguousarray(x.reshape(T, D).astype(np.float32, copy=False))

    g_sel, w2 = _route(xf, np.asarray(macro_w), np.asarray(micro_w))
    idx_by_g = [np.nonzero(g_sel == g)[0] for g in range(NUM_GROUPS)]
    n_e = [idx_by_g[e // 2].size for e in range(NUM_EXPERTS)]

    s0, s1, nsets, ab = _plan(n_e)
    segs = [(0, s0), (s0, s1)]
    nc = _get_program(s0, s1, nsets)

    gate_b = np.ascontiguousarray(np.asarray(gate_w, np.float32)).astype(ml_dtypes.bfloat16)
    up_b = np.ascontiguousarray(np.asarray(up_w, np.float32)).astype(ml_dtypes.bfloat16)
    down_b = np.ascontiguousarray(np.asarray(down_w, np.float32)).astype(ml_dtypes.bfloat16)

    # hand out buckets: free lists of (core, seg)
    free = [[(c, si) for c in range(N_CORES)] for si in range(2)]
    # jobs[(core, seg)] = (expert, token_indices)
    jobs = {}
    for e in range(NUM_EXPERTS):
        a, b = ab[e]
        ix = idx_by_g[e // 2]
        pos = 0
        for si, cnt in ((0, a), (1, b)):
            cap = (s0, s1)[si]
            for _ in range(cnt):
                c, _si = free[si].pop(0)
                take = ix[pos:pos + cap]
                pos += cap
                jobs[(c, si)] = (e, take)
    # unassigned buckets: dummy expert 0 with no tokens
    for si in range(2):
        for c, _ in free[si]:
            jobs[(c, si)] = (0, np.empty(0, np.int64))

    C = s0 + s1
    xfb = xf.astype(ml_dtypes.bfloat16)
    in_maps = []
    for c in range(N_CORES):
        xt = np.zeros((D, C), ml_dtypes.bfloat16)
        m = {"xt": xt}
        for si, (off, slen) in enumerate(segs):
            e, ix = jobs[(c, si)]
            if ix.size:
                xt[:, off:off + ix.size] = xfb[ix].T
            k = si if nsets == 2 else 0
            if si == 0 or nsets == 2:
                m[f"gw{k}"] = gate_b[e]
                m[f"uw{k}"] = up_b[e]
                m[f"dw{k}"] = down_b[e]
        in_maps.append(m)

    res = run_bass_kernel_spmd(nc, in_maps, core_ids=list(range(N_CORES)))
    LAST_RESULTS = res

    y = np.zeros((T, D), np.float32)
    for c in range(N_CORES):
        ytc = res.results[c]["yt"]
        for si, (off, slen) in enumerate(segs):
            e, ix = jobs[(c, si)]
            if ix.size:
                y[ix] += w2[ix, e % 2][:, None] * ytc[:, off:off + ix.size].T
    return y.reshape(B, S, D)


# revision 6
# speedup vs baseline: 1.1102x; 1.1102x over previous
"""MoE FFN (grouped top-1 routing, SwiGLU experts) on 8 Trainium2 NeuronCores.

Strategy (expert-parallel with static 2-segment load balancing):
  - Host computes the (tiny) routers in float64: sigmoid(x @ macro_w) -> top-1
    group of 4; within the selected group both 2 experts are active
    (TOP_K == EXPERTS_PER_GROUP) with sigmoid-normalized weights.
  - Each core processes C = s0 + s1 token columns in two statically-sized
    segments.  Each segment has its own full SwiGLU weight-set input, so the
    host can assign ANY expert to any (core, segment) bucket.  A small search
    picks (s0, s1) so the 8 expert token-loads pack into the 16 buckets with
    minimal C (544 vs 608 for naive per-expert capacity on the benchmark
    routing distribution).
  - The per-token routing weight is applied on the HOST to the fp32 partial
    outputs (y[t] = w0*y_e0[t] + w1*y_e1[t]), so the device never sees it and
    x is shipped only once.
  - Device kernel per segment: Y^T = down^T @ (silu(gate^T X^T) * (up^T X^T)),
    features on SBUF partitions, tokens on the free dim, bf16 storage/matmuls,
    fp32 PSUM accumulation.  All weight DMAs are issued in exact consumption
    order on the sync HWDGE queue (x first) so the PE never starves and the
    HAM clock-gate warms once.
"""

import numpy as np
import ml_dtypes

import concourse.bass as bass  # noqa: F401  (bass types via bacc)
import concourse.mybir as mybir
import concourse.tile as tile
from concourse import bacc
from concourse.bass_utils import run_bass_kernel_spmd

P = 128
D_MODEL = 1024
FFN_DIM = 2048
NUM_EXPERTS = 8
NUM_GROUPS = 4
EPS = 1e-9
DO = D_MODEL // P   # 8 k-tiles over D
FO = FFN_DIM // P   # 16 f-tiles over F

F32 = mybir.dt.float32
BF16 = mybir.dt.bfloat16

N_CORES = 8

_BUILD_CACHE: dict[tuple, object] = {}
_PLAN_CACHE: dict[tuple, tuple] = {}
LAST_RESULTS = None  # stashed BassKernelResults for test harnesses


# ──────────────────────────────────────────────────────────────────────
# Device program
# ──────────────────────────────────────────────────────────────────────

def _build(s0: int, s1: int, nsets: int = 2):
    """Bass/Tile program: C=s0+s1 token columns, two segments, each with its
    own full SwiGLU weight set.  Segment A runs fully (gate/up then down),
    then segment B — so the weight stream only has to feed one segment's
    demand at a time (~250 GB/s vs the ~358 GB/s per-core HBM cap) and the
    PE never starves after the start transient."""
    C = s0 + s1
    segs = [(0, s0), (s0, s1)]

    nc = bacc.Bacc(
        "TRN2",
        target_bir_lowering=False,
        debug=False,
        enable_asserts=False,
        num_devices=N_CORES,
    )
    xt = nc.dram_tensor("xt", [D_MODEL, C], BF16, kind="ExternalInput").ap()
    gws, uws, dws = [], [], []
    for k in range(2):
        gws.append(nc.dram_tensor(f"gw{k}", [D_MODEL, FFN_DIM], BF16,
                                  kind="ExternalInput").ap())
        uws.append(nc.dram_tensor(f"uw{k}", [D_MODEL, FFN_DIM], BF16,
                                  kind="ExternalInput").ap())
        dws.append(nc.dram_tensor(f"dw{k}", [FFN_DIM, D_MODEL], BF16,
                                  kind="ExternalInput").ap())
    yt = nc.dram_tensor("yt", [D_MODEL, C], F32, kind="ExternalOutput").ap()

    gwr = [g.rearrange("(do p) f -> p do f", p=P) for g in gws]
    uwr = [u.rearrange("(do p) f -> p do f", p=P) for u in uws]
    dwr = [w.rearrange("(fo p) d -> p fo d", p=P) for w in dws]
    xtr = xt.rearrange("(do p) c -> p do c", p=P)

    # gate/up f-blocks (start_fo, n_fo): segment A leads with smaller blocks
    # so the PE can start sooner; B uses uniform wide blocks.
    blocksA = [(0, 2), (2, 2), (4, 4), (8, 4), (12, 4)]
    blocksB = [(0, 4), (4, 4), (8, 4), (12, 4)]
    ND = 2  # d-tiles per down-weight block

    with tile.TileContext(nc) as tc:
        with (
            tc.tile_pool(name="xp", bufs=1) as xp,
            tc.tile_pool(name="wp", bufs=4) as wp,
            tc.tile_pool(name="dp", bufs=4) as dp,
            tc.tile_pool(name="hp", bufs=1) as hp,
            tc.tile_pool(name="sp", bufs=4) as sp,
            tc.tile_pool(name="yp", bufs=4) as yp,
            tc.tile_pool(name="pgu", bufs=6, space="PSUM") as pgu,
            tc.tile_pool(name="pd", bufs=2, space="PSUM") as pd,
        ):
            xts = xp.tile([P, DO, C], BF16, tag="xt")
            gts, uts, dts = {}, {}, {}

            def load_gu(k, blocks):
                for b, (sfo, nfo) in enumerate(blocks):
                    fsl = slice(sfo * P, (sfo + nfo) * P)
                    g = wp.tile([P, DO, nfo * P], BF16, tag="gt", name=f"g{k}_{b}")
                    nc.sync.dma_start(g[:], gwr[k][:, :, fsl])
                    u = wp.tile([P, DO, nfo * P], BF16, tag="ut", name=f"u{k}_{b}")
                    nc.sync.dma_start(u[:], uwr[k][:, :, fsl])
                    gts[(k, b)] = g
                    uts[(k, b)] = u

            def load_dw(k):
                for db in range(DO // ND):
                    dsl = slice(db * ND * P, (db + 1) * ND * P)
                    dt_ = dp.tile([P, FO, ND * P], BF16, tag="dt", name=f"d{k}_{db}")
                    nc.sync.dma_start(dt_[:], dwr[k][:, :, dsl])
                    dts[(k, db)] = dt_

            # ALL loads on the single sync HWDGE ring in strict consumption
            # order (two rings just time-slice the same ~358 GB/s and starve
            # the critical prefix).  Critical prefix: xA, gate-b0 — the first
            # two gate fo-groups run before any up weights are needed.
            nc.sync.dma_start(xts[:, :, 0:s0], xtr[:, :, 0:s0])
            load_gu(0, blocksA)
            load_dw(0)
            nc.sync.dma_start(xts[:, :, s0:C], xtr[:, :, s0:C])
            load_gu(1, blocksB)
            load_dw(1)

            hs = hp.tile([P, FO, C], BF16, tag="h")

            def gate_up_fo(si, off, slen, fo, blocks, which=None):
                k = si
                csl = slice(off, off + slen)
                b = None
                for bb, (sfo, nfo) in enumerate(blocks):
                    if sfo <= fo < sfo + nfo:
                        b = bb
                        break
                fl = fo - blocks[b][0]
                fsl = slice(fl * P, (fl + 1) * P)
                if which in (None, "gate"):
                    psg = pgu.tile([P, slen], F32, tag="ps", name=f"psg_{fo}_{si}")
                    gate_up_fo.psg[(fo, si)] = psg
                    for do in range(DO):
                        nc.tensor.matmul(
                            psg[:], gts[(k, b)][:, do, fsl], xts[:, do, csl],
                            start=(do == 0), stop=(do == DO - 1),
                        )
                if which in (None, "up"):
                    psu = pgu.tile([P, slen], F32, tag="ps", name=f"psu_{fo}_{si}")
                    for do in range(DO):
                        nc.tensor.matmul(
                            psu[:], uts[(k, b)][:, do, fsl], xts[:, do, csl],
                            start=(do == 0), stop=(do == DO - 1),
                        )
                    psg = gate_up_fo.psg.pop((fo, si))
                    sg = sp.tile([P, slen], F32, tag="sg")
                    nc.scalar.activation(
                        sg[:], psg[:], mybir.ActivationFunctionType.Silu
                    )
                    nc.vector.tensor_mul(out=hs[:, fo, csl], in0=sg[:], in1=psu[:])
            gate_up_fo.psg = {}

            for si, (off, slen) in enumerate(segs):
                k = si
                csl = slice(off, off + slen)
                blocks = blocksA if si == 0 else blocksB

                # gate/up for this segment.  For segment A, run the two
                # gate fo-groups of block 0 before any up group so the PE
                # starts as soon as (xA, gate-b0) have landed.
                if si == 0:
                    gate_up_fo(si, off, slen, 0, blocks, "gate")
                    gate_up_fo(si, off, slen, 1, blocks, "gate")
                    gate_up_fo(si, off, slen, 0, blocks, "up")
                    gate_up_fo(si, off, slen, 1, blocks, "up")
                    fo_start = 2
                else:
                    fo_start = 0
                for fo in range(fo_start, FO):
                    gate_up_fo(si, off, slen, fo, blocks)

                # down for this segment
                for db in range(DO // ND):
                    for half in range(ND):
                        do = db * ND + half
                        dsl = slice(half * P, (half + 1) * P)
                        psy = pd.tile([P, slen], F32, tag="psy",
                                      name=f"psy_{do}_{si}")
                        for fo in range(FO):
                            nc.tensor.matmul(
                                psy[:], dts[(k, db)][:, fo, dsl], hs[:, fo, csl],
                                start=(fo == 0), stop=(fo == FO - 1),
                            )
                        yo = yp.tile([P, slen], F32, tag="yo")
                        nc.any.tensor_copy(out=yo[:], in_=psy[:])
                        nc.scalar.dma_start(yt[do * P:(do + 1) * P, csl], yo[:])
    nc.finalize()
    return nc


def _get_program(s0: int, s1: int, nsets: int = 2):
    key = (s0, s1)
    if key not in _BUILD_CACHE:
        _BUILD_CACHE[key] = _build(s0, s1)
    return _BUILD_CACHE[key]


# ──────────────────────────────────────────────────────────────────────
# Host routing
# ──────────────────────────────────────────────────────────────────────

def _sigmoid(z):
    return 1.0 / (1.0 + np.exp(-z))


def _route(xf32, macro_w, micro_w):
    """Host routers in float64. Returns group index per token and per-token
    weights for the 2 experts of the selected group (float32)."""
    xf = xf32.astype(np.float64)
    ms = _sigmoid(xf @ macro_w.astype(np.float64))  # [T, G]
    g_sel = np.argmax(ms, axis=1)
    T = xf.shape[0]
    mval = ms[np.arange(T), g_sel]
    mv = mval / (mval + EPS)

    w2 = np.zeros((T, 2), np.float64)
    for g in range(NUM_GROUPS):
        idx = np.nonzero(g_sel == g)[0]
        if idx.size == 0:
            continue
        s = _sigmoid(xf[idx] @ micro_w[g].astype(np.float64))  # [n, 2]
        denom = s[:, 0] + s[:, 1] + EPS
        w2[idx, 0] = mv[idx] * s[:, 0] / denom
        w2[idx, 1] = mv[idx] * s[:, 1] / denom
    return g_sel, w2.astype(np.float32)


# ──────────────────────────────────────────────────────────────────────
# Segment-size search + bucket assignment
# ──────────────────────────────────────────────────────────────────────

def _feasible(n_e, s0, s1):
    """Can loads n_e pack into 8 buckets of s0 and 8 of s1 (each bucket a
    single expert)?  Returns per-expert (a, b) bucket counts or None."""
    cands = []
    for n in n_e:
        cc = []
        if n == 0:
            cc.append((0, 0))
        else:
            for a in range(9):
                rem = n - a * s0
                b = 0 if rem <= 0 else -(-rem // s1)
                if b <= 8:
                    cc.append((a, b))
            # prune dominated
            cc.sort()
            pruned, best_b = [], 99
            for a, b in cc:
                if b < best_b:
                    pruned.append((a, b))
                    best_b = b
            cc = pruned
        cands.append(cc)
    # DP over (sum_a, sum_b)
    states = {(0, 0): []}
    for cc in cands:
        nxt = {}
        for (ua, ub), hist in states.items():
            for a, b in cc:
                na, nb = ua + a, ub + b
                if na <= 8 and nb <= 8 and (na, nb) not in nxt:
                    nxt[(na, nb)] = hist + [(a, b)]
        states = nxt
        if not states:
            return None
    return next(iter(states.values()))


def _plan(n_e):
    """Pick (s0, s1, nsets, ab) minimizing C = s0 + s1."""
    key = tuple(n_e)
    if key in _PLAN_CACHE:
        return _PLAN_CACHE[key]
    best = None
    for s0 in range(64, 513, 16):
        for s1 in range(48, s0 + 1, 16):
            ab = _feasible(n_e, s0, s1)
            if ab is None:
                continue
            c = s0 + s1
            if best is None or c < best[0] or (c == best[0] and s0 < best[1]):
                best = (c, s0, s1, ab)
    if best is None:
        raise RuntimeError(f"no feasible segment plan for loads {n_e}")
    _, s0, s1, ab = best
    # single-expert-per-core case needs only one weight set
    nsets = 2
    if all(a + b <= (1 if n else 99) for (a, b), n in zip(ab, n_e)) and False:
        nsets = 1  # (kept for experiments; default always 2)
    plan = (s0, s1, nsets, ab)
    _PLAN_CACHE[key] = plan
    return plan


# ──────────────────────────────────────────────────────────────────────
# Entry point
# ──────────────────────────────────────────────────────────────────────

def kernel(x, macro_w, micro_w, gate_w, up_w, down_w):
    global LAST_RESULTS
    x = np.asarray(x)
    B, S, D = x.shape
    T = B * S
    xf = np.ascontiguousarray(x.reshape(T, D).astype(np.float32, copy=False))

    g_sel, w2 = _route(xf, np.asarray(macro_w), np.asarray(micro_w))
    idx_by_g = [np.nonzero(g_sel == g)[0] for g in range(NUM_GROUPS)]
    n_e = [idx_by_g[e // 2].size for e in range(NUM_EXPERTS)]

    s0, s1, nsets, ab = _plan(n_e)
    segs = [(0, s0), (s0, s1)]
    nc = _get_program(s0, s1, nsets)

    gate_b = np.ascontiguousarray(np.asarray(gate_w, np.float32)).astype(ml_dtypes.bfloat16)
    up_b = np.ascontiguousarray(np.asarray(up_w, np.float32)).astype(ml_dtypes.bfloat16)
    down_b = np.ascontiguousarray(np.asarray(down_w, np.float32)).astype(ml_dtypes.bfloat16)

    # hand out buckets: free lists of (core, seg)
    free = [[(c, si) for c in range(N_CORES)] for si in range(2)]
    # jobs[(core, seg)] = (expert, token_indices)
    jobs = {}
    for e in range(NUM_EXPERTS):
        a, b = ab[e]
        ix = idx_by_g[e // 2]
        pos = 0
        for si, cnt in ((0, a), (1, b)):
            cap = (s0, s1)[si]
            for _ in range(cnt):
                c, _si = free[si].pop(0)
                take = ix[pos:pos + cap]
                pos += cap
                jobs[(c, si)] = (e, take)
    # unassigned buckets: dummy expert 0 with no tokens
    for si in range(2):
        for c, _ in free[si]:
            jobs[(c, si)] = (0, np.empty(0, np.int64))

    C = s0 + s1
    xfb = xf.astype(ml_dtypes.bfloat16)
    in_maps = []
    for c in range(N_CORES):
        xt = np.zeros((D, C), ml_dtypes.bfloat16)
        m = {"xt": xt}
        for si, (off, slen) in enumerate(segs):
            e, ix = jobs[(c, si)]
            if ix.size:
                xt[:, off:off + ix.size] = xfb[ix].T
            k = si if nsets == 2 else 0
            if si == 0 or nsets == 2:
                m[f"gw{k}"] = gate_b[e]
                m[f"uw{k}"] = up_b[e]
                m[f"dw{k}"] = down_b[e]
        in_maps.append(m)

    res = run_bass_kernel_spmd(nc, in_maps, core_ids=list(range(N_CORES)))
    LAST_RESULTS = res

    y = np.zeros((T, D), np.float32)
    for c in range(N_CORES):
        ytc = res.results[c]["yt"]
        for si, (off, slen) in enumerate(segs):
            e, ix = jobs[(c, si)]
            if ix.size:
                y[ix] += w2[ix, e % 2][:, None] * ytc[:, off:off + ix.size].T
    return y.reshape(B, S, D)
